# revision 1
# baseline (speedup 1.0000x reference)
"""GQA attention block (B=2, T=2048, C=2048, H=32, Hkv=8, D=64, RoPE, causal)
on 8 TRN2 NeuronCores.

Sharding: core = b*4 + g  (b = batch 0..1, g = head-group 0..3).
Each core computes 8 Q heads / 2 KV heads of one batch element:
  QKV projections -> RoPE -> causal softmax(QK^T/sqrt(D)) V -> partial
  output projection against its 512 columns of Wc.  Host sums the 4
  head-group partials per batch.

Pipeline structure (per core): one fused loop over the 4 sequence blocks.
Iteration tb projects Q/K/V for t-block tb, then runs attention for
q-block tb (which only needs K/V up to block tb), then the output
projection for those rows.  Projection matmuls (PE-heavy) overlap the
previous block's attention (ScalarE-exp-heavy) in the Tile schedule.

Attention computes S^T = K Q^T tiles (k on partitions) so the exp'd
tiles feed the PV matmul with no transposes; a ones-column appended to V
accumulates the softmax denominator in the same matmul; causal masking
skips fully-masked tiles, narrows diagonal-crossing streams, and applies
a 128x128 triangle mask to the diagonal block.  The two heads sharing a
128-partition tile (Q pair (p, p+4), KV pair) pack into PE row groups
via base partitions 0/64 and share one [128, 2, 512] score PSUM tile so
each exp covers both heads in one ACTIVATE.

Matmul operands are bf16 by default (KERNEL_MM_DTYPE=f32r selects
float32r: slower, ~15x lower error); PSUM accumulation is always fp32.
"""

import os

import ml_dtypes
import numpy as np

import concourse.bacc as bacc
import concourse.mybir as mybir
from concourse.tile import TileContext
from concourse.bass_utils import run_bass_kernel_spmd

B, T, C = 2, 2048, 2048
H, HKV, D = 32, 8, 64
ROPE_THETA = 10000.0

P = 128
NCT = C // P          # 16 contraction subtiles
TB = 512              # t-block width
NTB = T // TB         # 4
QB = 512              # q-block width in attention
KT = T // P           # 16 k-tiles
QH = H // 4           # 8 local q heads per core
LOCAL_HEADS = [0, 4, 1, 5, 2, 6, 3, 7]  # pair (p, p+4) shares a 128-row tile

F32 = mybir.dt.float32
F32R = mybir.dt.float32r
BF16 = mybir.dt.bfloat16

MM_MODE = os.environ.get("KERNEL_MM_DTYPE", "bf16")
MMDT = BF16 if MM_MODE == "bf16" else F32R
NPDT = ml_dtypes.bfloat16 if MM_MODE == "bf16" else np.float32

EXP_SCALE = float(1.0 / np.sqrt(D))


def build_bass():
    nc = bacc.Bacc("TRN2", target_bir_lowering=False, debug=False, num_devices=8)

    xT = nc.dram_tensor("xT", [C, T], MMDT, kind="ExternalInput")
    wqT = nc.dram_tensor("wqT", [C, QH * D], MMDT, kind="ExternalInput")
    wkT = nc.dram_tensor("wkT", [C, 2 * D], MMDT, kind="ExternalInput")
    wvT = nc.dram_tensor("wvT", [C, 2 * D], MMDT, kind="ExternalInput")
    wcT = nc.dram_tensor("wcT", [QH * D, C], MMDT, kind="ExternalInput")
    cosT = nc.dram_tensor("cosT", [P, T], F32, kind="ExternalInput")
    sinT = nc.dram_tensor("sinT", [P, T], F32, kind="ExternalInput")
    tri = nc.dram_tensor("tri", [P, P], MMDT, kind="ExternalInput")
    ident = nc.dram_tensor("ident", [P, P], MMDT, kind="ExternalInput")
    vones = nc.dram_tensor("vones", [P, KT, 2], MMDT, kind="ExternalInput")
    out = nc.dram_tensor("out", [T, C], F32, kind="ExternalOutput")

    xT_r = xT.rearrange("(o p) t -> p o t", p=P)      # [128, 16, T]
    wqT_r = wqT.rearrange("(o p) m -> p o m", p=P)    # [128, 16, 512]
    wkT_r = wkT.rearrange("(o p) m -> p o m", p=P)    # [128, 16, 128]
    wvT_r = wvT.rearrange("(o p) m -> p o m", p=P)
    wcT_r = wcT.rearrange("(o p) c -> p o c", p=P)    # [128, 4, 2048]

    with TileContext(nc) as tc:
        with (
            tc.tile_pool(name="persist", bufs=1) as persist,
            tc.tile_pool(name="small", bufs=4) as small,
            tc.tile_pool(name="xs", bufs=8) as xs,
            tc.tile_pool(name="rot", bufs=3) as rotp,
            tc.tile_pool(name="vt", bufs=2) as vtp,
            tc.tile_pool(name="pt", bufs=6) as ptp,
            tc.tile_pool(name="pvc", bufs=3) as pvcp,
            tc.tile_pool(name="ostage", bufs=4) as ostage,
            tc.tile_pool(name="psMM", bufs=2, space="PSUM") as psMM,
            tc.tile_pool(name="psST", bufs=2, space="PSUM") as psST,
            tc.tile_pool(name="psPV", bufs=1, space="PSUM") as psPV,
        ):
            # ---- persistent SBUF tensors ------------------------------
            q_sb = persist.tile([P, 4, T], MMDT)          # Q^T (rope'd)
            k_sb = persist.tile([P, T], MMDT)             # K^T (rope'd)
            v_sb = persist.tile([P, KT, 2, D + 1], MMDT)  # V + ones col
            y_sb = persist.tile([P, 4, T], MMDT)          # attn out^T
            tri_sb = persist.tile([P, P], MMDT)
            id_sb = persist.tile([P, P], MMDT)
            cos_sb = persist.tile([P, T], F32)
            sin_sb = persist.tile([P, T], F32)
            wk_sb = persist.tile([P, NCT, 2 * D], MMDT, tag="wk")
            wv_sb = persist.tile([P, NCT, 2 * D], MMDT, tag="wv")
            wq_sb = persist.tile([P, NCT, QH * D], MMDT, tag="wq")
            wc_sb = persist.tile([P, 4, C], MMDT, tag="wc")

            nc.sync.dma_start(wk_sb[:], wkT_r[:])

            def rope_store(dst, psum, tb):
                # dst/psum: [128, TB]; q_rope = q*cos + rot(q)*sin'
                tmp = rotp.tile([P, TB], F32, tag="rp_t")
                rtmp = rotp.tile([P, TB], F32, tag="rp_r")
                nc.scalar.copy(tmp[:], psum[:])
                for olo, ilo in ((0, 32), (32, 0), (64, 96), (96, 64)):
                    nc.sync.dma_start(
                        rtmp[olo : olo + 32, :], tmp[ilo : ilo + 32, :]
                    )
                ts = slice(tb * TB, (tb + 1) * TB)
                nc.vector.tensor_mul(dst, tmp[:], cos_sb[:, ts])
                nc.vector.tensor_mul(rtmp[:], rtmp[:], sin_sb[:, ts])
                nc.vector.tensor_add(dst, dst, rtmp[:])

            def proj_block(tb):
                tsl = slice(tb * TB, (tb + 1) * TB)
                # ---- x^T stream (two halves of the contraction dim) ---
                xh = []
                for qtr in range(4):
                    xb = xs.tile(
                        [P, NCT // 4, TB], MMDT, tag="xb", name=f"xb{qtr}"
                    )
                    nc.sync.dma_start(
                        xb[:],
                        xT_r[:, qtr * (NCT // 4) : (qtr + 1) * (NCT // 4), tsl],
                    )
                    xh.append(xb)

                def xc(c):
                    return xh[c // (NCT // 4)][:, c % (NCT // 4), :]

                if tb == 0:
                    nc.sync.dma_start(wq_sb[:], wqT_r[:])
                    nc.sync.dma_start(wv_sb[:], wvT_r[:])
                    nc.sync.dma_start(cos_sb[:], cosT[:])
                    nc.sync.dma_start(sin_sb[:], sinT[:])
                    nc.sync.dma_start(tri_sb[:], tri[:])
                    nc.sync.dma_start(id_sb[:], ident[:])
                    nc.sync.dma_start(v_sb[:, :, :, D], vones[:])
                    nc.gpsimd.dma_start(wc_sb[:], wcT_r[:])

                # ---- K^T projection (one [128, TB] tile: 2 kv heads) --
                pk = psMM.tile([P, TB], F32, tag="mm512", name="pk")
                for c in range(NCT):
                    nc.tensor.matmul(
                        pk[:], wk_sb[:, c, :], xc(c),
                        start=(c == 0), stop=(c == NCT - 1),
                    )
                rope_store(k_sb[:, tsl], pk, tb)

                # ---- Q^T m-tiles (4 x [128, TB]) ----------------------
                for m in range(4):
                    pq = psMM.tile([P, TB], F32, tag="mm512", name="pq")
                    for c in range(NCT):
                        nc.tensor.matmul(
                            pq[:], wq_sb[:, c, m * P : (m + 1) * P], xc(c),
                            start=(c == 0), stop=(c == NCT - 1),
                        )
                    rope_store(q_sb[:, m, tsl], pq, tb)

                # ---- V: project V^T then PE-transpose to [t, d] -------
                pvt = psMM.tile([P, TB], F32, tag="mm512", name="pvt")
                for c in range(NCT):
                    nc.tensor.matmul(
                        pvt[:], wv_sb[:, c, :], xc(c),
                        start=(c == 0), stop=(c == NCT - 1),
                    )
                vt_sb = vtp.tile([P, TB], MMDT, tag="vt")
                nc.vector.tensor_copy(vt_sb[:], pvt[:])
                for s in range(TB // P):
                    kt = tb * (TB // P) + s
                    ptr = psMM.tile([P, P], MMDT, tag="mm512", name="ptr")
                    nc.tensor.transpose(
                        ptr[:], vt_sb[:, s * P : (s + 1) * P], id_sb[:]
                    )
                    for hh in range(2):
                        nc.vector.tensor_copy(
                            v_sb[:, kt, hh, 0:D],
                            ptr[:, hh * D : (hh + 1) * D],
                        )

            def attn_block(jq):
                qb = jq * QB
                nkt = 4 * jq + 4
                for pr in range(4):  # head-pair tiles (local heads pr, pr+4)
                    pv = psPV.tile([D + 1, 2, QB], F32, tag="pv")
                    for kt in range(nkt):
                        j = kt - 4 * jq  # >= 0: diagonal-crossing tile
                        w = QB - P * j if j >= 0 else QB
                        qoff = qb + P * j if j >= 0 else qb
                        ksl = slice(kt * P, (kt + 1) * P)
                        st = psST.tile([P, 2, QB], F32, tag="st")
                        for hh in range(2):
                            hsl = slice(hh * D, (hh + 1) * D)
                            nc.tensor.matmul(
                                st[:, hh, 0:w],
                                k_sb[hsl, ksl],
                                q_sb[hsl, pr, qoff : qoff + w],
                                start=True,
                                stop=True,
                            )
                        ptile = ptp.tile([P, 2, QB], MMDT, tag="pt")
                        nc.scalar.activation(
                            ptile[:, :, 0:w],
                            st[:, :, 0:w],
                            mybir.ActivationFunctionType.Exp,
                            scale=EXP_SCALE,
                        )
                        if j >= 0:
                            nc.vector.tensor_mul(
                                ptile[:, :, 0:P],
                                ptile[:, :, 0:P],
                                tri_sb[:, None, :].to_broadcast((P, 2, P)),
                            )
                        for hh in range(2):
                            nc.tensor.matmul(
                                pv[:, hh, qoff - qb :],
                                v_sb[:, kt, hh, :],
                                ptile[:, hh, 0:w],
                                start=(kt == 0),
                                stop=(kt == nkt - 1),
                            )
                    # move PV psum to SBUF, then normalize by the sum row
                    pvc = pvcp.tile([D + 1, 2, QB], F32, tag="pvc")
                    nc.vector.tensor_copy(pvc[:], pv[:])
                    for hh in range(2):
                        srow = small.tile([1, QB], F32, tag="srow")
                        nc.vector.tensor_copy(srow[:], pv[D : D + 1, hh, :])
                        rec = small.tile([1, QB], F32, tag="rec")
                        nc.vector.reciprocal_approx_fast(rec[:], srow[:])
                        bc = small.tile([D, QB], F32, tag="bc")
                        nc.gpsimd.partition_broadcast(bc[:], rec[:])
                        nc.vector.tensor_mul(
                            y_sb[hh * D : (hh + 1) * D, pr, qb : qb + QB],
                            pvc[0:D, hh, :],
                            bc[:],
                        )

            def outproj_block(jq):
                # ---- output projection for rows jq*TB..(jq+1)*TB ------
                for cb in range(4):
                    csl = slice(cb * 512, (cb + 1) * 512)
                    for s in range(4):
                        t = jq * 4 + s
                        po = psMM.tile([P, 512], F32, tag="mm512", name="po")
                        for jj in range(4):
                            nc.tensor.matmul(
                                po[:],
                                y_sb[:, jj, t * P : (t + 1) * P],
                                wc_sb[:, jj, csl],
                                start=(jj == 0),
                                stop=(jj == 3),
                            )
                        ob = ostage.tile([P, 512], F32, tag="ob")
                        if s % 2 == 0:
                            nc.scalar.copy(ob[:], po[:])
                        else:
                            nc.vector.tensor_copy(ob[:], po[:])
                        nc.gpsimd.dma_start(out[t * P : (t + 1) * P, csl], ob[:])

            # emission order: proj(tb+1) before out-proj(tb) so blocked
            # out-proj PSUM tiles don't starve the projection pipeline
            proj_block(0)
            for tb in range(NTB):
                attn_block(tb)
                if tb + 1 < NTB:
                    proj_block(tb + 1)
                outproj_block(tb)

    nc.finalize()
    return nc


def _rope_tables(position_ids):
    t = position_ids.reshape(-1).astype(np.float64)  # [T]
    inv_freq = 1.0 / ROPE_THETA ** (np.arange(0, D, 2, dtype=np.float64) / D)
    freqs = np.outer(t, inv_freq)  # [T, D/2]
    cos = np.repeat(np.cos(freqs), 2, axis=1)  # [T, D] interleaved
    sin = np.repeat(np.sin(freqs), 2, axis=1)
    sign = np.where(np.arange(D) < D // 2, -1.0, 1.0)
    cosT = np.tile(cos.T, (2, 1)).astype(np.float32)            # [128, T]
    sinT = np.tile((sin * sign).T, (2, 1)).astype(np.float32)   # [128, T]
    return np.ascontiguousarray(cosT), np.ascontiguousarray(sinT)


def _head_perm(g):
    # row indices into Wq (and columns of Wc) for core head-group g
    rows = []
    for lh in LOCAL_HEADS:
        h = g * QH + lh
        rows.extend(range(h * D, (h + 1) * D))
    return np.asarray(rows)


def make_in_maps(x, Wq, Wk, Wv, Wc, position_ids):
    x = np.asarray(x, dtype=np.float32)
    Wq = np.asarray(Wq, dtype=np.float32)
    Wk = np.asarray(Wk, dtype=np.float32)
    Wv = np.asarray(Wv, dtype=np.float32)
    Wc = np.asarray(Wc, dtype=np.float32)
    cosT, sinT = _rope_tables(np.asarray(position_ids))
    tri = np.triu(np.ones((P, P), dtype=np.float32))  # allow q >= k
    in_maps = []
    for core in range(8):
        b, g = divmod(core, 4)
        perm = _head_perm(g)
        kv = slice(2 * g * D, (2 * g + 2) * D)
        in_maps.append(
            {
                "xT": np.ascontiguousarray(x[b].T).astype(NPDT),
                "wqT": np.ascontiguousarray(Wq[perm].T).astype(NPDT),
                "wkT": np.ascontiguousarray(Wk[kv].T).astype(NPDT),
                "wvT": np.ascontiguousarray(Wv[kv].T).astype(NPDT),
                "wcT": np.ascontiguousarray(Wc[:, perm].T).astype(NPDT),
                "cosT": cosT,
                "sinT": sinT,
                "tri": tri.astype(NPDT),
                "ident": np.eye(P, dtype=np.float32).astype(NPDT),
                "vones": np.ones((P, KT, 2), dtype=NPDT),
            }
        )
    return in_maps


_NC = None


def get_nc():
    global _NC
    if _NC is None:
        _NC = build_bass()
    return _NC


def run_cores(in_maps, core_ids, **kw):
    return run_bass_kernel_spmd(get_nc(), in_maps, core_ids=core_ids, **kw)


def kernel(x, Wq, Wk, Wv, Wc, position_ids, _trace=False, _res_out=None):
    in_maps = make_in_maps(x, Wq, Wk, Wv, Wc, position_ids)
    res = run_cores(in_maps, list(range(8)), trace=_trace)
    if _res_out is not None:
        _res_out.append(res)
    outs = [res.results[i]["out"] for i in range(8)]
    y = np.stack(
        [
            outs[0] + outs[1] + outs[2] + outs[3],
            outs[4] + outs[5] + outs[6] + outs[7],
        ]
    )
    return y.astype(np.float32)

